# revision 1
# baseline (speedup 1.0000x reference)
"""Trainium2 Bass kernel for an 8-layer Mamba stack (nn_NewMamba).

Sharding: data-parallel over batch (16 -> 8 cores x 2).
Layout: activations kept as [channel(partitions), time(free)] per batch elem.
Scan: hardware tensor_tensor_scan (state = dA*state + x) along the free dim,
one recurrence per (i, s) pair; S-contraction via interleaved layout +
masked-scan segmented sum.
"""

import numpy as np

import concourse.bass as bass
import concourse.mybir as mybir
import concourse.tile as tile
from concourse.bass import ds, ts
from concourse.masks import make_identity

FP32 = mybir.dt.float32
BF16 = mybir.dt.bfloat16
AF = mybir.ActivationFunctionType
OP = mybir.AluOpType

H = 256       # hidden
I = 512       # intermediate
S = 16        # ssm state
R = 16        # time step rank
KCONV = 4     # conv kernel
NL = 8        # layers
EPS = 1e-5
B = 16
LFULL = 2048
NCORES = 8
BLOC = B // NCORES   # 2
P = 128
HC = H // P          # 2
ICN = I // P         # 4
OCN = 2 * I // P     # 8
NT = 512             # matmul free-dim tile


def build_program(L=LFULL, n_layers=NL):
    NT = min(512, L)          # matmul free-dim tile
    assert L % P == 0 and L % NT == 0
    TH = min(256, L)          # ssm time chunk (scan + y-stage granularity)
    NTH = L // TH
    nc = bass.Bass()

    # ---- external I/O ----
    x_in = nc.declare_dram_parameter("x", [BLOC, L, H], FP32, isOutput=False)
    norm_w = nc.declare_dram_parameter("norm_w", [NL, H], FP32, isOutput=False)
    in_w = nc.declare_dram_parameter("in_proj_w", [NL, 2 * I, H], FP32, isOutput=False)
    conv_w = nc.declare_dram_parameter("conv_w", [NL, I, KCONV], FP32, isOutput=False)
    conv_b = nc.declare_dram_parameter("conv_b", [NL, I], FP32, isOutput=False)
    xp_w = nc.declare_dram_parameter("x_proj_w", [NL, R + 2 * S, I], FP32, isOutput=False)
    dt_w = nc.declare_dram_parameter("dt_proj_w", [NL, I, R], FP32, isOutput=False)
    dt_b = nc.declare_dram_parameter("dt_proj_b", [NL, I], FP32, isOutput=False)
    A_log = nc.declare_dram_parameter("A_log", [NL, I, S], FP32, isOutput=False)
    D_in = nc.declare_dram_parameter("D", [NL, I], FP32, isOutput=False)
    out_w = nc.declare_dram_parameter("out_proj_w", [NL, H, I], FP32, isOutput=False)
    y_out = nc.declare_dram_parameter("out", [BLOC, L, H], FP32, isOutput=True)

    # ---- dram scratch ----
    xT_dram = nc.dram_tensor("xT_scr", [BLOC, HC, P, L], FP32)
    w_inT = nc.dram_tensor("w_inT_scr", [n_layers, HC, P, 2 * I], BF16)
    w_outT = nc.dram_tensor("w_outT_scr", [n_layers, ICN, P, H], BF16)
    XP80 = 80
    w_xpT = nc.dram_tensor("w_xpT_scr", [n_layers, ICN, P, 80], BF16)
    w_dtT = nc.dram_tensor("w_dtT_scr", [n_layers, R + 1, I], BF16)
    r_dram = nc.dram_tensor("r_scr", [1, L], BF16)
    gate_dram = nc.dram_tensor("gate_scr", [ICN, P, L], BF16)
    B_dram = nc.dram_tensor("B_scr", [S, L], BF16)
    C_dram = nc.dram_tensor("C_scr", [S, L], BF16)

    with tile.TileContext(nc) as tc:
        with (
            tc.tile_pool(name="glob", bufs=1) as pg,
            tc.tile_pool(name="prep", bufs=1) as pw,
            tc.tile_pool(name="layer", bufs=1) as pl,
            tc.tile_pool(name="trans", bufs=1) as pt,
            tc.tile_pool(name="ssm", bufs=2) as ps,
            tc.tile_pool(name="crep", bufs=2) as pc,
            tc.tile_pool(name="brep", bufs=2) as pb,
            tc.tile_pool(name="hint", bufs=1) as ph,
            tc.tile_pool(name="psum", bufs=3, space="PSUM") as pp,
            tc.tile_pool(name="psumT", bufs=1, space="PSUM") as ppt,
            tc.tile_pool(name="psum1", bufs=1, space="PSUM") as pp1,
        ):
            # ---- global constants ----
            ident = pg.tile([P, P], FP32, name="ident")
            make_identity(nc, ident)
            ones_col = pg.tile([P, 1], BF16, name="ones_col")
            nc.vector.memset(ones_col, 1.0)
            eps_col = pg.tile([P, 1], FP32, name="eps_col")
            nc.vector.memset(eps_col, EPS)
            one_col = pg.tile([P, 1], FP32, name="one_col")
            nc.vector.memset(one_col, 1.0)
            # ---- weight prep (once) ----
            for li in range(n_layers):
                # in_proj: want lhsT [H, 2I] = in_w.T
                winT_sb = [pw.tile([P, 2 * I], BF16, name=f"winT_sb{h}") for h in range(HC)]
                for oc in range(OCN):
                    wtile = pw.tile([P, I], FP32, name="w_ld2")[:, :H]
                    nc.sync.dma_start(wtile, in_w[li, ts(oc, P), :])
                    for hc in range(HC):
                        pst = ppt.tile([P, P], FP32, name="pst")
                        nc.tensor.matmul(pst, wtile[:, ts(hc, P)], ident)
                        nc.scalar.copy(winT_sb[hc][:, ts(oc, P)], pst)
                for hc in range(HC):
                    nc.sync.dma_start(w_inT[li, hc], winT_sb[hc])
                # out_proj: want lhsT [I, H] = out_w.T
                woutT_sb = [pw.tile([P, H], BF16, name=f"woutT_sb{c}") for c in range(ICN)]
                for hc in range(HC):
                    wtile = pw.tile([P, I], FP32, name="w_ld2")
                    nc.sync.dma_start(wtile, out_w[li, ts(hc, P), :])
                    for ic in range(ICN):
                        pst = ppt.tile([P, P], FP32, name="pst")
                        nc.tensor.matmul(pst, wtile[:, ts(ic, P)], ident)
                        nc.scalar.copy(woutT_sb[ic][:, ts(hc, P)], pst)
                for ic in range(ICN):
                    nc.sync.dma_start(w_outT[li, ic], woutT_sb[ic])
                # x_proj: want lhsT [I, 48] = xp_w.T
                xp_sb = pw.tile([R + 2 * S, I], FP32, name="w_ld2")
                nc.sync.dma_start(xp_sb, xp_w[li])
                for ic in range(ICN):
                    pst = ppt.tile([P, P], FP32, name="pst")
                    nc.tensor.matmul(
                        pst[:, : R + 2 * S], xp_sb[:, ts(ic, P)],
                        ident[: R + 2 * S, : R + 2 * S],
                    )
                    wx = pw.tile([P, XP80], BF16, name="wx")
                    nc.vector.memset(wx, 0.0)
                    nc.scalar.copy(wx[:, :R], pst[:, :R])        # dt rows 0:16
                    nc.scalar.copy(wx[:, 32:48], pst[:, R : R + S])       # B -> 32:48
                    nc.scalar.copy(wx[:, 64:80], pst[:, R + S : R + 2 * S])  # C -> 64:80
                    nc.sync.dma_start(w_xpT[li, ic], wx)
                # dt_proj: want lhsT [R+1, I]: rows 0..R-1 = dt_w.T, row R = dt_b
                wdt32 = pw.tile([R + 1, I], FP32, name="w_ld2")
                for ic in range(ICN):
                    wtile = pw.tile([P, R], FP32, name="w_ld3")
                    nc.sync.dma_start(wtile, dt_w[li, ts(ic, P), :])
                    pst = ppt.tile([P, P], FP32, name="pst")
                    nc.tensor.matmul(pst[:R], wtile, ident)
                    nc.scalar.copy(wdt32[:R, ts(ic, P)], pst[:R])
                nc.sync.dma_start(wdt32[R : R + 1, :], dt_b[li][None, :])
                wdt_sb = pw.tile([R + 1, I], BF16, name="wdt_sb")
                nc.vector.tensor_copy(wdt_sb, wdt32)
                nc.sync.dma_start(w_dtT[li], wdt_sb)

            # ---- transpose x into [H, L] layout in dram scratch ----
            for b in range(BLOC):
                xT_sb = [pw.tile([P, L], FP32, name=f"xT_io{h}") for h in range(HC)]
                for tc_i in range(L // P):
                    xt_ld = pw.tile([P, H], FP32, name="xio_small")
                    nc.sync.dma_start(xt_ld, x_in[b, ts(tc_i, P), :])
                    for hc in range(HC):
                        pst = ppt.tile([P, P], FP32, name="pst")
                        nc.tensor.matmul(pst, xt_ld[:, ts(hc, P)], ident)
                        nc.vector.tensor_copy(xT_sb[hc][:, ts(tc_i, P)], pst)
                for hc in range(HC):
                    nc.sync.dma_start(xT_dram[b, hc], xT_sb[hc])

            # ================= layers =================
            for li in range(n_layers):
                # per-layer small tensors
                w_in_sb = [pl.tile([P, 2 * I], BF16, name=f"w_in{h}") for h in range(HC)]
                for hc in range(HC):
                    nc.sync.dma_start(w_in_sb[hc], w_inT[li, hc])
                w_out_sb = [pl.tile([P, H], BF16, name=f"w_out{c}") for c in range(ICN)]
                w_xp_sb = [pl.tile([P, XP80], BF16, name=f"w_xp{c}") for c in range(ICN)]
                for ic in range(ICN):
                    nc.sync.dma_start(w_out_sb[ic], w_outT[li, ic])
                    nc.sync.dma_start(w_xp_sb[ic], w_xpT[li, ic])
                w_dt_sb = pl.tile([R + 1, I], BF16, name="w_dt")
                nc.sync.dma_start(w_dt_sb, w_dtT[li])
                cw_sb = [pl.tile([P, KCONV], FP32, name=f"cw{c}") for c in range(ICN)]
                cb_sb = [pl.tile([P, 1], FP32, name=f"cb{c}") for c in range(ICN)]
                d_sb = [pl.tile([P, 1], FP32, name=f"dsk{c}") for c in range(ICN)]
                a_neg = [pl.tile([P, S], FP32, name=f"an{c}") for c in range(ICN)]
                a_bias = [pl.tile([P, S], FP32, name=f"ab{c}") for c in range(ICN)]
                for ic in range(ICN):
                    nc.sync.dma_start(cw_sb[ic], conv_w[li, ts(ic, P), :])
                    nc.sync.dma_start(cb_sb[ic], conv_b[li, ts(ic, P)][:, None])
                    nc.sync.dma_start(d_sb[ic], D_in[li, ts(ic, P)][:, None])
                    atile = pt.tile([P, S], FP32, name="a_ld")
                    nc.sync.dma_start(atile, A_log[li, ts(ic, P), :])
                    nc.scalar.activation(a_neg[ic], atile, AF.Exp)
                    nc.vector.tensor_scalar_mul(a_neg[ic], a_neg[ic], -1.0)
                    nc.vector.tensor_scalar_mul(a_bias[ic], a_neg[ic], 0.7)
                nw_sb = [pl.tile([P, 1], FP32, name=f"nw{h}") for h in range(HC)]
                for hc in range(HC):
                    nc.sync.dma_start(nw_sb[hc], norm_w[li, ts(hc, P)][:, None])

                for b in range(BLOC):
                    # ---- load x (residual stream) in [H, L] layout ----
                    xT = [pl.tile([P, L], FP32, name=f"xT{h}") for h in range(HC)]
                    for hc in range(HC):
                        nc.sync.dma_start(xT[hc], xT_dram[b, hc])

                    # ---- rmsnorm ----
                    ms_ps = pp1.tile([1, L], FP32, name="ms_ps")
                    sq = [pt.tile([P, L], BF16, name=f"sq{h}") for h in range(HC)]
                    for hc in range(HC):
                        nc.scalar.activation(sq[hc], xT[hc], AF.Square)
                    for nn in range(L // NT):
                        for hc in range(HC):
                            nc.tensor.matmul(
                                ms_ps[:, ts(nn, NT)], ones_col, sq[hc][:, ts(nn, NT)],
                                start=(hc == 0), stop=(hc == HC - 1),
                            )
                    nc.scalar.activation(ms_ps, ms_ps, AF.Sqrt, bias=eps_col[:1], scale=1.0 / H)
                    nc.vector.reciprocal(ms_ps, ms_ps)
                    r16 = pt.tile([1, L], BF16, name="r16")
                    nc.vector.tensor_copy(r16, ms_ps)
                    nc.sync.dma_start(r_dram.ap(), r16)
                    r_rep = pt.tile([P, L], BF16, name="r_rep")
                    nc.sync.dma_start(r_rep, r_dram.ap().to_broadcast((P, L)))
                    hn = [pt.tile([P, L], BF16, name=f"hn{h}") for h in range(HC)]
                    for hc in range(HC):
                        xb = pt.tile([P, L], BF16, name=f"sq{hc}")
                        nc.vector.tensor_copy(xb, xT[hc])
                        nc.vector.scalar_tensor_tensor(
                            hn[hc], xb, nw_sb[hc], r_rep, op0=OP.mult, op1=OP.mult
                        )

                    # ---- in_proj ----
                    hs_pad = [pl.tile([P, KCONV - 1 + L], BF16, name=f"hsp{c}") for c in range(ICN)]
                    for ic in range(ICN):
                        nc.vector.memset(hs_pad[ic][:, 0 : KCONV - 1], 0.0)
                    for oc in range(OCN):
                        for nn in range(L // NT):
                            psm = pp.tile([P, NT], FP32, name="psm")
                            for hc in range(HC):
                                nc.tensor.matmul(
                                    psm, w_in_sb[hc][:, ts(oc, P)], hn[hc][:, ts(nn, NT)],
                                    start=(hc == 0), stop=(hc == HC - 1),
                                )
                            if oc < ICN:
                                nc.scalar.copy(
                                    hs_pad[oc][:, KCONV - 1 + nn * NT : KCONV - 1 + (nn + 1) * NT],
                                    psm,
                                )
                            else:
                                gtmp = pt.tile([P, NT], BF16, name="gtmp")
                                nc.scalar.activation(gtmp, psm, AF.Silu)
                                nc.sync.dma_start(gate_dram[oc - ICN, :, ts(nn, NT)], gtmp)

                    # ---- depthwise causal conv + silu ----
                    u = [pl.tile([P, L], BF16, name=f"u{c}") for c in range(ICN)]
                    for ic in range(ICN):
                        cacc = pt.tile([P, L], BF16, name="cacc")
                        nc.vector.tensor_scalar_mul(cacc, hs_pad[ic][:, 0:L], cw_sb[ic][:, 0:1])
                        for k in range(1, KCONV):
                            nc.vector.scalar_tensor_tensor(
                                cacc, hs_pad[ic][:, k : k + L], cw_sb[ic][:, k : k + 1],
                                cacc, op0=OP.mult, op1=OP.add,
                            )
                        nc.scalar.activation(u[ic], cacc, AF.Silu, bias=cb_sb[ic])

                    # ---- x_proj (normal order) fused with dt_proj ----
                    dtp = [pl.tile([P, L], BF16, name=f"dtp{c}") for c in range(ICN)]
                    for nn in range(L // NT):
                        ps48_f = pp.tile([P, NT], FP32, name="psm")
                        ps48 = ps48_f[:XP80]
                        for ic in range(ICN):
                            nc.tensor.matmul(
                                ps48, w_xp_sb[ic], u[ic][:, ts(nn, NT)],
                                start=(ic == 0), stop=(ic == ICN - 1),
                            )
                        dtr_nn = pt.tile([R + 1, NT], BF16, name="dtr_nn")
                        nc.vector.memset(dtr_nn, 1.0)  # row R = ones (bias row)
                        nc.scalar.copy(dtr_nn[0:R], ps48[0:R])
                        bt = pt.tile([S, NT], BF16, name="bt")
                        nc.scalar.copy(bt, ps48[32:48])
                        nc.sync.dma_start(B_dram.ap()[:, ts(nn, NT)], bt)
                        ct = pt.tile([S, NT], BF16, name="ct")
                        nc.scalar.copy(ct, ps48[64:80])
                        nc.sync.dma_start(C_dram.ap()[:, ts(nn, NT)], ct)
                        for mc in range(ICN):
                            psd = pp.tile([P, NT], FP32, name="psm")
                            nc.tensor.matmul(psd, w_dt_sb[:, ts(mc, P)], dtr_nn)
                            dt32 = pt.tile([P, NT], FP32, name="dt32")
                            nc.scalar.activation(dt32, psd, AF.Exp)
                            # softplus(x) = ln(exp(x) + 1)
                            nc.scalar.activation(dt32, dt32, AF.Ln, bias=one_col)
                            nc.vector.tensor_scalar_add(dtp[mc][:, ts(nn, NT)], dt32, -0.7)

                    # ---- dtu ----
                    dtu = [pl.tile([P, L], BF16, name=f"dtu{c}") for c in range(ICN)]
                    for mc in range(ICN):
                        nc.vector.scalar_tensor_tensor(
                            dtu[mc], dtp[mc], 0.7, u[mc], op0=OP.add, op1=OP.mult
                        )

                    # ---- SSM scan: s-major, full-L contiguous ----
                    y_acc = [pl.tile([P, L], BF16, name=f"hsp{c}") for c in range(ICN)]
                    for s in range(S):
                        B_rep = pb.tile([P, L], BF16, name="B_rep")
                        nc.sync.dma_start(
                            B_rep, B_dram.ap()[s : s + 1, :].to_broadcast((P, L))
                        )
                        C_rep = pc.tile([P, L], BF16, name="C_rep")
                        nc.sync.dma_start(
                            C_rep, C_dram.ap()[s : s + 1, :].to_broadcast((P, L))
                        )
                        for ic in range(ICN):
                            dA = ps.tile([P, L], BF16, name="dA")
                            nc.scalar.activation(
                                dA, dtp[ic], AF.Exp,
                                bias=a_bias[ic][:, s : s + 1],
                                scale=a_neg[ic][:, s : s + 1],
                            )
                            xt = ps.tile([P, L], BF16, name="xt")
                            nc.vector.tensor_tensor(xt, dtu[ic], B_rep, op=OP.mult)
                            hscan = ps.tile([P, L], BF16, name="hscan")
                            nc.vector.tensor_tensor_scan(
                                hscan, dA, xt, 0.0, op0=OP.mult, op1=OP.add
                            )
                            if s == 0:
                                nc.vector.tensor_tensor(
                                    y_acc[ic], hscan, C_rep, op=OP.mult
                                )
                            else:
                                nc.vector.tensor_tensor(xt, hscan, C_rep, op=OP.mult)
                                nc.vector.tensor_tensor(
                                    y_acc[ic], y_acc[ic], xt, op=OP.add
                                )
                    y_ssm = y_acc

                    # ---- combine + out_proj + residual ----
                    for ic in range(ICN):
                        nc.vector.scalar_tensor_tensor(
                            y_ssm[ic], u[ic], d_sb[ic], y_ssm[ic], op0=OP.mult, op1=OP.add
                        )
                        for nn in range(L // NT):
                            gld = pt.tile([P, NT], BF16, name="gld")
                            nc.sync.dma_start(gld, gate_dram[ic, :, ts(nn, NT)])
                            nc.vector.tensor_tensor(
                                y_ssm[ic][:, ts(nn, NT)], y_ssm[ic][:, ts(nn, NT)],
                                gld, op=OP.mult,
                            )
                    for hc in range(HC):
                        for nn in range(L // NT):
                            pso = pp.tile([P, NT], FP32, name="psm")
                            for ic in range(ICN):
                                nc.tensor.matmul(
                                    pso, w_out_sb[ic][:, ts(hc, P)], y_ssm[ic][:, ts(nn, NT)],
                                    start=(ic == 0), stop=(ic == ICN - 1),
                                )
                            nc.vector.tensor_tensor(
                                xT[hc][:, ts(nn, NT)], xT[hc][:, ts(nn, NT)], pso, op=OP.add
                            )
                    for hc in range(HC):
                        nc.sync.dma_start(xT_dram[b, hc], xT[hc])

            # ---- transpose x back to [L, H] and write out ----
            for b in range(BLOC):
                xT_fin = [pw.tile([P, L], FP32, name=f"xT_io{h}") for h in range(HC)]
                for hc in range(HC):
                    nc.sync.dma_start(xT_fin[hc], xT_dram[b, hc])
                for tc_i in range(L // P):
                    o_sb = pw.tile([P, H], FP32, name="xio_small")
                    for hc in range(HC):
                        pst = ppt.tile([P, P], FP32, name="pst")
                        nc.tensor.matmul(pst, xT_fin[hc][:, ts(tc_i, P)], ident)
                        nc.vector.tensor_copy(o_sb[:, ts(hc, P)], pst)
                    nc.sync.dma_start(y_out[b, ts(tc_i, P), :], o_sb)

    return nc




def _split_matmul_waits(nc):
    """walrus codegen allows limited sync waits per instruction;
    hoist extras into EventSemaphore instructions on the same engine."""
    ctr = 0
    for fn in nc.m.functions:
        for bb in fn.blocks:
            insts = bb.instructions
            out = []
            changed = False
            for inst in insts:
                si = inst.sync_info
                if (
                    not isinstance(inst, mybir.InstEventSemaphore)
                    and si is not None
                    and si.on_wait
                    and len(si.on_wait) > 1
                ):
                    waits = list(si.on_wait)
                    for w in waits[: -1]:
                        ev = mybir.InstEventSemaphore(
                            name=f"I-mmwait-{ctr}",
                            engine=inst.engine,
                            sync_info=mybir.SyncInfo(on_wait=[w], on_update=[]),
                            ins=[],
                            outs=[],
                        )
                        ctr += 1
                        out.append(ev)
                    inst.sync_info = mybir.SyncInfo(
                        on_wait=[waits[-1]], on_update=list(si.on_update or [])
                    )
                    changed = True
                out.append(inst)
            if changed:
                bb.instructions = out
    return nc


def kernel(**inputs):
    from concourse.bass_utils import run_bass_kernel_spmd

    x = np.asarray(inputs["x"], dtype=np.float32)
    Bfull, L, _ = x.shape
    nc = build_program(L=L, n_layers=NL)
    _split_matmul_waits(nc)

    weight_names = [
        "norm_w", "in_proj_w", "conv_w", "conv_b", "x_proj_w",
        "dt_proj_w", "dt_proj_b", "A_log", "D", "out_proj_w",
    ]
    weights = {k: np.asarray(inputs[k], dtype=np.float32) for k in weight_names}

    in_maps = []
    for c in range(NCORES):
        m = {"x": x[c * BLOC : (c + 1) * BLOC]}
        m.update(weights)
        in_maps.append(m)

    res = run_bass_kernel_spmd(nc, in_maps, core_ids=list(range(NCORES)))
    out = np.concatenate([r["out"] for r in res.results], axis=0)
    return out



# revision 6
# speedup vs baseline: 1.1302x; 1.1302x over previous
"""Trainium2 Bass kernel for an 8-layer Mamba stack (nn_NewMamba).

Sharding: data-parallel over batch (16 -> 8 cores x 2).
Layout: activations kept as [channel(partitions), time(free)] per batch elem.
Scan: hardware tensor_tensor_scan (state = dA*state + x) along the free dim,
one recurrence per (i, s) pair, chunked over time with carried initial state.
All reductions/accumulations (y over s, depthwise conv, D-skip, residual)
run on the tensor engine as PSUM-accumulated matmuls with identity/diagonal
stationary operands, keeping the vector engine free for scans and the
irreducible elementwise products.
"""

import numpy as np

import concourse.bass as bass
import concourse.mybir as mybir
import concourse.tile as tile
from concourse.bass import ds, ts
from concourse.masks import make_identity

FP32 = mybir.dt.float32
BF16 = mybir.dt.bfloat16
AF = mybir.ActivationFunctionType
OP = mybir.AluOpType

H = 256       # hidden
I = 512       # intermediate
S = 16        # ssm state
R = 16        # time step rank
KCONV = 4     # conv kernel
NL = 8        # layers
EPS = 1e-5
B = 16
LFULL = 2048
NCORES = 8
BLOC = B // NCORES   # 2
P = 128
HC = H // P          # 2
ICN = I // P         # 4
OCN = 2 * I // P     # 8
NT = 512             # matmul free-dim tile


def build_program(L=LFULL, n_layers=NL):
    NT = min(512, L)          # matmul free-dim tile / ssm chunk
    assert L % P == 0 and L % NT == 0
    NN = L // NT
    nc = bass.Bass()

    # ---- external I/O ----
    x_in = nc.declare_dram_parameter("x", [BLOC, L, H], FP32, isOutput=False)
    norm_w = nc.declare_dram_parameter("norm_w", [NL, H], FP32, isOutput=False)
    in_w = nc.declare_dram_parameter("in_proj_w", [NL, 2 * I, H], FP32, isOutput=False)
    conv_w = nc.declare_dram_parameter("conv_w", [NL, I, KCONV], FP32, isOutput=False)
    conv_b = nc.declare_dram_parameter("conv_b", [NL, I], FP32, isOutput=False)
    xp_w = nc.declare_dram_parameter("x_proj_w", [NL, R + 2 * S, I], FP32, isOutput=False)
    dt_w = nc.declare_dram_parameter("dt_proj_w", [NL, I, R], FP32, isOutput=False)
    dt_b = nc.declare_dram_parameter("dt_proj_b", [NL, I], FP32, isOutput=False)
    A_log = nc.declare_dram_parameter("A_log", [NL, I, S], FP32, isOutput=False)
    D_in = nc.declare_dram_parameter("D", [NL, I], FP32, isOutput=False)
    out_w = nc.declare_dram_parameter("out_proj_w", [NL, H, I], FP32, isOutput=False)
    y_out = nc.declare_dram_parameter("out", [BLOC, L, H], FP32, isOutput=True)

    # ---- dram scratch ----
    xT_dram = nc.dram_tensor("xT_scr", [BLOC, HC, P, L], FP32)
    w_inT = nc.dram_tensor("w_inT_scr", [n_layers, HC, P, 2 * I], BF16)
    w_outT = nc.dram_tensor("w_outT_scr", [n_layers, ICN, P, H], BF16)
    XP80 = 80
    w_xpT = nc.dram_tensor("w_xpT_scr", [n_layers, ICN, P, 80], BF16)
    w_dtT = nc.dram_tensor("w_dtT_scr", [n_layers, R + 1, I], BF16)
    r_dram = nc.dram_tensor("r_scr", [1, L], BF16)
    B_dram = nc.dram_tensor("B_scr", [S, L], BF16)
    C_dram = nc.dram_tensor("C_scr", [S, L], BF16)

    with tile.TileContext(nc) as tc:
        with (
            tc.tile_pool(name="glob", bufs=1) as pg,
            tc.tile_pool(name="prep", bufs=1) as pw,
            tc.tile_pool(name="layer", bufs=1) as pl,
            tc.tile_pool(name="act", bufs=1) as pa,
            tc.tile_pool(name="trans", bufs=1) as pt,
            tc.tile_pool(name="ssm", bufs=3) as ps,
            tc.tile_pool(name="brep", bufs=2) as pb,
            tc.tile_pool(name="psum", bufs=3, space="PSUM") as pp,
            tc.tile_pool(name="psumy", bufs=1, space="PSUM") as ppy,
            tc.tile_pool(name="psumT", bufs=1, space="PSUM") as ppt,
        ):
            # ---- global constants ----
            ident = pg.tile([P, P], FP32, name="ident")
            make_identity(nc, ident)
            ident_bf = pg.tile([P, P], BF16, name="ident_bf")
            nc.vector.tensor_copy(ident_bf, ident)
            ones_col = pg.tile([P, 1], BF16, name="ones_col")
            nc.vector.memset(ones_col, 1.0)
            eps_col = pg.tile([P, 1], FP32, name="eps_col")
            nc.vector.memset(eps_col, EPS)
            one_col = pg.tile([P, 1], FP32, name="one_col")
            nc.vector.memset(one_col, 1.0)
            # ---- weight prep (once) ----
            for li in range(n_layers):
                # norm_w folded into in_proj: lhsT [H, 2I] = (in_w * nw).T
                nw_col = [pw.tile([P, 1], FP32, name=f"nw_pre{h}") for h in range(HC)]
                for hc in range(HC):
                    nc.sync.dma_start(nw_col[hc], norm_w[li, ts(hc, P)][:, None])
                winT_sb = [pw.tile([P, 2 * I], BF16, name=f"winT_sb{h}") for h in range(HC)]
                for oc in range(OCN):
                    wtile = pw.tile([P, I], FP32, name="w_ld2")[:, :H]
                    nc.sync.dma_start(wtile, in_w[li, ts(oc, P), :])
                    for hc in range(HC):
                        pst = ppt.tile([P, P], FP32, name="pst")
                        nc.tensor.matmul(pst, wtile[:, ts(hc, P)], ident)
                        # rows of lhsT are H-dim -> scale row h by norm_w[h]
                        nc.vector.tensor_scalar_mul(
                            winT_sb[hc][:, ts(oc, P)], pst, nw_col[hc]
                        )
                for hc in range(HC):
                    nc.sync.dma_start(w_inT[li, hc], winT_sb[hc])
                # out_proj: want lhsT [I, H] = out_w.T
                woutT_sb = [pw.tile([P, H], BF16, name=f"woutT_sb{c}") for c in range(ICN)]
                for hc in range(HC):
                    wtile = pw.tile([P, I], FP32, name="w_ld2")
                    nc.sync.dma_start(wtile, out_w[li, ts(hc, P), :])
                    for ic in range(ICN):
                        pst = ppt.tile([P, P], FP32, name="pst")
                        nc.tensor.matmul(pst, wtile[:, ts(ic, P)], ident)
                        nc.scalar.copy(woutT_sb[ic][:, ts(hc, P)], pst)
                for ic in range(ICN):
                    nc.sync.dma_start(w_outT[li, ic], woutT_sb[ic])
                # x_proj: want lhsT [I, 48] = xp_w.T (padded layout 0:16,32:48,64:80)
                xp_sb = pw.tile([R + 2 * S, I], FP32, name="w_ld2")
                nc.sync.dma_start(xp_sb, xp_w[li])
                for ic in range(ICN):
                    pst = ppt.tile([P, P], FP32, name="pst")
                    nc.tensor.matmul(
                        pst[:, : R + 2 * S], xp_sb[:, ts(ic, P)],
                        ident[: R + 2 * S, : R + 2 * S],
                    )
                    wx = pw.tile([P, XP80], BF16, name="wx")
                    nc.vector.memset(wx, 0.0)
                    nc.scalar.copy(wx[:, :R], pst[:, :R])        # dt rows 0:16
                    nc.scalar.copy(wx[:, 32:48], pst[:, R : R + S])       # B -> 32:48
                    nc.scalar.copy(wx[:, 64:80], pst[:, R + S : R + 2 * S])  # C -> 64:80
                    nc.sync.dma_start(w_xpT[li, ic], wx)
                # dt_proj: want lhsT [R+1, I]: rows 0..R-1 = dt_w.T, row R = dt_b
                wdt32 = pw.tile([R + 1, I], FP32, name="w_ld2")
                for ic in range(ICN):
                    wtile = pw.tile([P, R], FP32, name="w_ld3")
                    nc.sync.dma_start(wtile, dt_w[li, ts(ic, P), :])
                    pst = ppt.tile([P, P], FP32, name="pst")
                    nc.tensor.matmul(pst[:R], wtile, ident)
                    nc.scalar.copy(wdt32[:R, ts(ic, P)], pst[:R])
                nc.sync.dma_start(wdt32[R : R + 1, :], dt_b[li][None, :])
                wdt_sb = pw.tile([R + 1, I], BF16, name="wdt_sb")
                nc.vector.tensor_copy(wdt_sb, wdt32)
                nc.sync.dma_start(w_dtT[li], wdt_sb)

            # ---- transpose x into [H, L] layout in dram scratch ----
            for b in range(BLOC):
                xT_sb = [pw.tile([P, L], FP32, name=f"xT_io{h}") for h in range(HC)]
                for tc_i in range(L // P):
                    xt_ld = pw.tile([P, H], FP32, name="xio_small")
                    nc.sync.dma_start(xt_ld, x_in[b, ts(tc_i, P), :])
                    for hc in range(HC):
                        pst = ppt.tile([P, P], FP32, name="pst")
                        nc.tensor.matmul(pst, xt_ld[:, ts(hc, P)], ident)
                        nc.vector.tensor_copy(xT_sb[hc][:, ts(tc_i, P)], pst)
                for hc in range(HC):
                    nc.sync.dma_start(xT_dram[b, hc], xT_sb[hc])

            # ================= layers =================
            for li in range(n_layers):
                # per-layer small tensors
                w_in_sb = [pl.tile([P, 2 * I], BF16, name=f"w_in{h}") for h in range(HC)]
                for hc in range(HC):
                    nc.sync.dma_start(w_in_sb[hc], w_inT[li, hc])
                w_out_sb = [pl.tile([P, H], BF16, name=f"w_out{c}") for c in range(ICN)]
                w_xp_sb = [pl.tile([P, XP80], BF16, name=f"w_xp{c}") for c in range(ICN)]
                for ic in range(ICN):
                    nc.sync.dma_start(w_out_sb[ic], w_outT[li, ic])
                    nc.sync.dma_start(w_xp_sb[ic], w_xpT[li, ic])
                w_dt_sb = pl.tile([R + 1, I], BF16, name="w_dt")
                nc.sync.dma_start(w_dt_sb, w_dtT[li])
                cb_sb = [pl.tile([P, 1], FP32, name=f"cb{c}") for c in range(ICN)]
                a_neg = [pl.tile([P, S], FP32, name=f"an{c}") for c in range(ICN)]
                a_bias = [pl.tile([P, S], FP32, name=f"ab{c}") for c in range(ICN)]
                # conv weights as diagonal [P,P] bf16 matmul operands
                cdiag = [
                    [pl.tile([P, P], BF16, name=f"cd{c}_{k}") for k in range(KCONV)]
                    for c in range(ICN)
                ]
                ddiag = [pl.tile([P, P], BF16, name=f"dd{c}") for c in range(ICN)]
                for ic in range(ICN):
                    cw_ld = pt.tile([P, KCONV], FP32, name="cw_ld")
                    nc.sync.dma_start(cw_ld, conv_w[li, ts(ic, P), :])
                    for k in range(KCONV):
                        nc.vector.tensor_scalar_mul(
                            cdiag[ic][k], ident_bf, cw_ld[:, k : k + 1]
                        )
                    d_ld = pt.tile([P, 1], FP32, name="d_ld")
                    nc.sync.dma_start(d_ld, D_in[li, ts(ic, P)][:, None])
                    nc.vector.tensor_scalar_mul(ddiag[ic], ident_bf, d_ld)
                    nc.sync.dma_start(cb_sb[ic], conv_b[li, ts(ic, P)][:, None])
                    atile = pt.tile([P, S], FP32, name="a_ld")
                    nc.sync.dma_start(atile, A_log[li, ts(ic, P), :])
                    nc.scalar.activation(a_neg[ic], atile, AF.Exp)
                    nc.vector.tensor_scalar_mul(a_neg[ic], a_neg[ic], -1.0)
                    nc.vector.tensor_scalar_mul(a_bias[ic], a_neg[ic], 0.7)

                for b in range(BLOC):
                    # ---- load x (residual stream) in [H, L] layout ----
                    xT = [pl.tile([P, L], FP32, name=f"xT{h}") for h in range(HC)]
                    for hc in range(HC):
                        nc.sync.dma_start(xT[hc], xT_dram[b, hc])

                    # ---- rmsnorm (norm_w folded into in_proj weights) ----
                    sq = [pt.tile([P, L], BF16, name=f"sq{h}") for h in range(HC)]
                    for hc in range(HC):
                        nc.scalar.activation(sq[hc], xT[hc], AF.Square)
                    r16 = pt.tile([1, L], BF16, name="r16")
                    for nn in range(NN):
                        ms_ps = pp.tile([P, NT], FP32, name="psm")[:1]
                        for hc in range(HC):
                            nc.tensor.matmul(
                                ms_ps, ones_col, sq[hc][:, ts(nn, NT)],
                                start=(hc == 0), stop=(hc == HC - 1),
                            )
                        # r = (mean_sq + eps)^-0.5 = exp(-0.5*ln(ms/H + eps))
                        lnr = pt.tile([1, NT], FP32, name="lnr")
                        nc.scalar.activation(
                            lnr, ms_ps, AF.Ln, bias=eps_col[:1], scale=1.0 / H
                        )
                        nc.scalar.activation(r16[:, ts(nn, NT)], lnr, AF.Exp, scale=-0.5)
                    nc.sync.dma_start(r_dram.ap(), r16)
                    r_rep = pt.tile([P, L], BF16, name="r_rep")
                    nc.sync.dma_start(r_rep, r_dram.ap().to_broadcast((P, L)))
                    hn = [pt.tile([P, L], BF16, name=f"hn{h}") for h in range(HC)]
                    for hc in range(HC):
                        xb = pt.tile([P, L], BF16, name=f"xb{hc}")
                        nc.vector.tensor_copy(xb, xT[hc])
                        nc.vector.tensor_tensor(hn[hc], xb, r_rep, op=OP.mult)

                    # ---- in_proj ----
                    hs_pad = [pl.tile([P, KCONV - 1 + L], BF16, name=f"hsp{c}") for c in range(ICN)]
                    sg = [pl.tile([P, L], BF16, name=f"sg{c}") for c in range(ICN)]
                    for ic in range(ICN):
                        nc.vector.memset(hs_pad[ic][:, 0 : KCONV - 1], 0.0)
                    for oc in range(OCN):
                        for nn in range(NN):
                            psm = pp.tile([P, NT], FP32, name="psm")
                            for hc in range(HC):
                                nc.tensor.matmul(
                                    psm, w_in_sb[hc][:, ts(oc, P)], hn[hc][:, ts(nn, NT)],
                                    start=(hc == 0), stop=(hc == HC - 1),
                                )
                            if oc < ICN:
                                nc.scalar.copy(
                                    hs_pad[oc][:, KCONV - 1 + nn * NT : KCONV - 1 + (nn + 1) * NT],
                                    psm,
                                )
                            else:
                                nc.scalar.activation(
                                    sg[oc - ICN][:, ts(nn, NT)], psm, AF.Silu
                                )

                    # ---- depthwise causal conv (tensor engine) + silu ----
                    u = [pl.tile([P, L], BF16, name=f"u{c}") for c in range(ICN)]
                    for ic in range(ICN):
                        for nn in range(NN):
                            pcv = pp.tile([P, NT], FP32, name="psm")
                            for k in range(KCONV):
                                nc.tensor.matmul(
                                    pcv, cdiag[ic][k],
                                    hs_pad[ic][:, k + nn * NT : k + (nn + 1) * NT],
                                    start=(k == 0), stop=(k == KCONV - 1),
                                )
                            nc.scalar.activation(
                                u[ic][:, ts(nn, NT)], pcv, AF.Silu, bias=cb_sb[ic]
                            )

                    # ---- x_proj (fused with dt_proj) ----
                    dtp = [pl.tile([P, L], BF16, name=f"dtp{c}") for c in range(ICN)]
                    for nn in range(NN):
                        ps48_f = pp.tile([P, NT], FP32, name="psm")
                        ps48 = ps48_f[:XP80]
                        for ic in range(ICN):
                            nc.tensor.matmul(
                                ps48, w_xp_sb[ic], u[ic][:, ts(nn, NT)],
                                start=(ic == 0), stop=(ic == ICN - 1),
                            )
                        dtr_nn = pt.tile([R + 1, NT], BF16, name="dtr_nn")
                        nc.vector.memset(dtr_nn, 1.0)  # row R = ones (bias row)
                        nc.scalar.copy(dtr_nn[0:R], ps48[0:R])
                        bt = pt.tile([S, NT], BF16, name="bt")
                        nc.scalar.copy(bt, ps48[32:48])
                        nc.sync.dma_start(B_dram.ap()[:, ts(nn, NT)], bt)
                        ct = pt.tile([S, NT], BF16, name="ct")
                        nc.scalar.copy(ct, ps48[64:80])
                        nc.sync.dma_start(C_dram.ap()[:, ts(nn, NT)], ct)
                        for mc in range(ICN):
                            psd = pp.tile([P, NT], FP32, name="psm")
                            nc.tensor.matmul(psd, w_dt_sb[:, ts(mc, P)], dtr_nn)
                            dt32 = pt.tile([P, NT], FP32, name="dt32")
                            nc.scalar.activation(dt32, psd, AF.Exp)
                            # softplus(x) = ln(exp(x) + 1); store centered at -0.7
                            nc.scalar.activation(dt32, dt32, AF.Ln, bias=one_col)
                            nc.vector.tensor_scalar_add(dtp[mc][:, ts(nn, NT)], dt32, -0.7)

                    # ---- dtu = dt * u ----
                    dtu = [pl.tile([P, L], BF16, name=f"dtu{c}") for c in range(ICN)]
                    for mc in range(ICN):
                        nc.vector.scalar_tensor_tensor(
                            dtu[mc], dtp[mc], 0.7, u[mc], op0=OP.add, op1=OP.mult
                        )

                    # ---- SSM scan, chunked over time; y accumulated on PE ----
                    y_ssm = [pl.tile([P, L], BF16, name=f"yss{c}") for c in range(ICN)]
                    carries = pl.tile([P, S * ICN], BF16, name="carries")
                    for nn in range(NN):
                        yps = [
                            ppy.tile([P, NT], FP32, name=f"yps{c}") for c in range(ICN)
                        ]
                        for ic in range(ICN):
                            # D-skip term starts the accumulation
                            nc.tensor.matmul(
                                yps[ic], ddiag[ic], u[ic][:, ts(nn, NT)],
                                start=True, stop=False,
                            )
                        for s in range(S):
                            B_rep = pb.tile([P, NT], BF16, name="B_rep")
                            nc.sync.dma_start(
                                B_rep,
                                B_dram.ap()[s : s + 1, ts(nn, NT)].to_broadcast((P, NT)),
                            )
                            C_rep = pb.tile([P, NT], BF16, name="C_rep")
                            nc.sync.dma_start(
                                C_rep,
                                C_dram.ap()[s : s + 1, ts(nn, NT)].to_broadcast((P, NT)),
                            )
                            for ic in range(ICN):
                                sic = s * ICN + ic
                                dA = ps.tile([P, NT], BF16, name="dA")
                                nc.scalar.activation(
                                    dA, dtp[ic][:, ts(nn, NT)], AF.Exp,
                                    bias=a_bias[ic][:, s : s + 1],
                                    scale=a_neg[ic][:, s : s + 1],
                                )
                                xt = ps.tile([P, NT], BF16, name="xt")
                                nc.vector.tensor_tensor(
                                    xt, dtu[ic][:, ts(nn, NT)], B_rep, op=OP.mult
                                )
                                hscan = ps.tile([P, NT], BF16, name="hscan")
                                nc.vector.tensor_tensor_scan(
                                    hscan, dA, xt,
                                    0.0 if nn == 0 else carries[:, sic : sic + 1],
                                    op0=OP.mult, op1=OP.add,
                                )
                                if nn < NN - 1:
                                    nc.vector.tensor_copy(
                                        carries[:, sic : sic + 1], hscan[:, NT - 1 : NT]
                                    )
                                hC = ps.tile([P, NT], BF16, name="hC")
                                nc.vector.tensor_tensor(hC, hscan, C_rep, op=OP.mult)
                                nc.tensor.matmul(
                                    yps[ic], ident_bf, hC,
                                    start=False, stop=(s == S - 1),
                                )
                        for ic in range(ICN):
                            # gate: y_ssm = ypsum * silu(gate)
                            nc.vector.tensor_tensor(
                                y_ssm[ic][:, ts(nn, NT)], yps[ic],
                                sg[ic][:, ts(nn, NT)], op=OP.mult,
                            )

                    # ---- out_proj + residual (PE) ----
                    for hc in range(HC):
                        for nn in range(NN):
                            pso = pp.tile([P, NT], FP32, name="psm")
                            nc.tensor.matmul(
                                pso, ident, xT[hc][:, ts(nn, NT)],
                                start=True, stop=False,
                            )
                            for ic in range(ICN):
                                nc.tensor.matmul(
                                    pso, w_out_sb[ic][:, ts(hc, P)], y_ssm[ic][:, ts(nn, NT)],
                                    start=False, stop=(ic == ICN - 1),
                                )
                            nc.scalar.copy(xT[hc][:, ts(nn, NT)], pso)
                    for hc in range(HC):
                        nc.sync.dma_start(xT_dram[b, hc], xT[hc])

            # ---- transpose x back to [L, H] and write out ----
            for b in range(BLOC):
                xT_fin = [pw.tile([P, L], FP32, name=f"xT_io{h}") for h in range(HC)]
                for hc in range(HC):
                    nc.sync.dma_start(xT_fin[hc], xT_dram[b, hc])
                for tc_i in range(L // P):
                    o_sb = pw.tile([P, H], FP32, name="xio_small")
                    for hc in range(HC):
                        pst = ppt.tile([P, P], FP32, name="pst")
                        nc.tensor.matmul(pst, xT_fin[hc][:, ts(tc_i, P)], ident)
                        nc.vector.tensor_copy(o_sb[:, ts(hc, P)], pst)
                    nc.sync.dma_start(y_out[b, ts(tc_i, P), :], o_sb)

    return nc




def _split_matmul_waits(nc):
    """walrus codegen allows limited sync waits per instruction;
    hoist extras into EventSemaphore instructions on the same engine."""
    ctr = 0
    for fn in nc.m.functions:
        for bb in fn.blocks:
            insts = bb.instructions
            out = []
            changed = False
            for inst in insts:
                si = inst.sync_info
                if (
                    not isinstance(inst, mybir.InstEventSemaphore)
                    and si is not None
                    and si.on_wait
                    and len(si.on_wait) > 1
                ):
                    waits = list(si.on_wait)
                    for w in waits[: -1]:
                        ev = mybir.InstEventSemaphore(
                            name=f"I-mmwait-{ctr}",
                            engine=inst.engine,
                            sync_info=mybir.SyncInfo(on_wait=[w], on_update=[]),
                            ins=[],
                            outs=[],
                        )
                        ctr += 1
                        out.append(ev)
                    inst.sync_info = mybir.SyncInfo(
                        on_wait=[waits[-1]], on_update=list(si.on_update or [])
                    )
                    changed = True
                out.append(inst)
            if changed:
                bb.instructions = out
    return nc


def kernel(**inputs):
    from concourse.bass_utils import run_bass_kernel_spmd

    x = np.asarray(inputs["x"], dtype=np.float32)
    Bfull, L, _ = x.shape
    nc = build_program(L=L, n_layers=NL)
    _split_matmul_waits(nc)

    weight_names = [
        "norm_w", "in_proj_w", "conv_w", "conv_b", "x_proj_w",
        "dt_proj_w", "dt_proj_b", "A_log", "D", "out_proj_w",
    ]
    weights = {k: np.asarray(inputs[k], dtype=np.float32) for k in weight_names}

    in_maps = []
    for c in range(NCORES):
        m = {"x": x[c * BLOC : (c + 1) * BLOC]}
        m.update(weights)
        in_maps.append(m)

    res = run_bass_kernel_spmd(nc, in_maps, core_ids=list(range(NCORES)))
    out = np.concatenate([r["out"] for r in res.results], axis=0)
    return out


# revision 11
# speedup vs baseline: 1.3209x; 1.1687x over previous
"""Trainium2 Bass kernel for an 8-layer Mamba stack (nn_NewMamba).

Sharding: data-parallel over batch (16 -> 8 cores x 2).
Layout: activations kept as [channel(partitions), time(free)] per batch elem.
Scan: hardware tensor_tensor_scan (state = dA*state + x) along the free dim,
one full-length recurrence per (i, s) pair. All reductions/accumulations
(y over s, depthwise conv, D-skip, residual) run on the tensor engine as
PSUM-accumulated matmuls with identity/diagonal stationary operands, keeping
the vector engine free for scans and the irreducible elementwise products.
"""

import numpy as np

import concourse.bass as bass
import concourse.mybir as mybir
import concourse.tile as tile
from concourse.bass import ds, ts
from concourse.masks import make_identity

FP32 = mybir.dt.float32
BF16 = mybir.dt.bfloat16
AF = mybir.ActivationFunctionType
OP = mybir.AluOpType

H = 256       # hidden
I = 512       # intermediate
S = 16        # ssm state
R = 16        # time step rank
KCONV = 4     # conv kernel
NL = 8        # layers
EPS = 1e-5
B = 16
LFULL = 2048
NCORES = 8
BLOC = B // NCORES   # 2
P = 128
HC = H // P          # 2
ICN = I // P         # 4
OCN = 2 * I // P     # 8
XP80 = 80


def build_program(L=LFULL, n_layers=NL):
    NT = min(512, L)          # matmul free-dim tile
    assert L % P == 0 and L % NT == 0
    NN = L // NT
    nc = bass.Bass()

    # ---- external I/O ----
    x_in = nc.declare_dram_parameter("x", [BLOC, L, H], FP32, isOutput=False)
    norm_w = nc.declare_dram_parameter("norm_w", [NL, H], FP32, isOutput=False)
    in_w = nc.declare_dram_parameter("in_proj_w", [NL, 2 * I, H], FP32, isOutput=False)
    conv_w = nc.declare_dram_parameter("conv_w", [NL, I, KCONV], FP32, isOutput=False)
    conv_b = nc.declare_dram_parameter("conv_b", [NL, I], FP32, isOutput=False)
    xp_w = nc.declare_dram_parameter("x_proj_w", [NL, R + 2 * S, I], FP32, isOutput=False)
    dt_w = nc.declare_dram_parameter("dt_proj_w", [NL, I, R], FP32, isOutput=False)
    dt_b = nc.declare_dram_parameter("dt_proj_b", [NL, I], FP32, isOutput=False)
    A_log = nc.declare_dram_parameter("A_log", [NL, I, S], FP32, isOutput=False)
    D_in = nc.declare_dram_parameter("D", [NL, I], FP32, isOutput=False)
    out_w = nc.declare_dram_parameter("out_proj_w", [NL, H, I], FP32, isOutput=False)
    y_out = nc.declare_dram_parameter("out", [BLOC, L, H], FP32, isOutput=True)

    # ---- dram scratch ----
    xT_dram = nc.dram_tensor("xT_scr", [BLOC, HC, P, L], BF16)
    w_inT = nc.dram_tensor("w_inT_scr", [n_layers, HC, P, 2 * I], BF16)
    w_outT = nc.dram_tensor("w_outT_scr", [n_layers, ICN, P, H], BF16)
    w_xpT = nc.dram_tensor("w_xpT_scr", [n_layers, ICN, P, XP80], BF16)
    w_dtT = nc.dram_tensor("w_dtT_scr", [n_layers, R + 1, I], BF16)
    r_dram = nc.dram_tensor("r_scr", [1, L], BF16)
    B_dram = nc.dram_tensor("B_scr", [S, L], BF16)
    C_dram = nc.dram_tensor("C_scr", [S, L], BF16)

    with tile.TileContext(nc) as tc:
        with (
            tc.tile_pool(name="glob", bufs=1) as pg,
            tc.tile_pool(name="prep", bufs=2) as pw,
            tc.tile_pool(name="prepbig", bufs=1) as pwb,
            tc.tile_pool(name="layer", bufs=1) as pl,
            tc.tile_pool(name="trans", bufs=1) as pt,
            tc.tile_pool(name="ssm", bufs=2) as ps,
            tc.tile_pool(name="brep", bufs=2) as pb,
            tc.tile_pool(name="psum", bufs=3, space="PSUM") as pp,
            tc.tile_pool(name="psumy", bufs=1, space="PSUM") as ppy,
            tc.tile_pool(name="psumT", bufs=1, space="PSUM") as ppt,
        ):
            # ---- global constants ----
            ident = pg.tile([P, P], FP32, name="ident")
            make_identity(nc, ident)
            ident_bf = pg.tile([P, P], BF16, name="ident_bf")
            nc.vector.tensor_copy(ident_bf, ident)
            ones_col = pg.tile([P, 1], BF16, name="ones_col")
            nc.vector.memset(ones_col, 1.0)
            eps_col = pg.tile([P, 1], FP32, name="eps_col")
            nc.vector.memset(eps_col, EPS)
            one_col = pg.tile([P, 1], FP32, name="one_col")
            nc.vector.memset(one_col, 1.0)

            # ---- transpose x into [H, L] layout in dram scratch (first: the
            # layer pipeline depends on it) ----
            for b in range(BLOC):
                for tc_i in range(L // P):
                    xt_ld = pw.tile([P, H], FP32, name="xio_small")
                    nc.sync.dma_start(xt_ld, x_in[b, ts(tc_i, P), :])
                    for hc in range(HC):
                        pst = ppt.tile([P, P], FP32, name="pst")
                        nc.tensor.matmul(pst, xt_ld[:, ts(hc, P)], ident)
                        o_sm = pw.tile([P, P], BF16, name="xio_o")
                        nc.vector.tensor_copy(o_sm, pst)
                        nc.sync.dma_start(xT_dram[b, hc][:, ts(tc_i, P)], o_sm)

            # ---- weight prep (once) ----
            for li in range(n_layers):
                # norm_w folded into in_proj: lhsT [H, 2I] = (in_w * nw).T
                nw_col = [pw.tile([P, 1], FP32, name=f"nw_pre{h}") for h in range(HC)]
                for hc in range(HC):
                    nc.sync.dma_start(nw_col[hc], norm_w[li, ts(hc, P)][:, None])
                winT_sb = [pwb.tile([P, 2 * I], BF16, name=f"winT_sb{h}") for h in range(HC)]
                for oc in range(OCN):
                    wtile = pw.tile([P, I], FP32, name="w_ld2")[:, :H]
                    nc.sync.dma_start(wtile, in_w[li, ts(oc, P), :])
                    for hc in range(HC):
                        pst = ppt.tile([P, P], FP32, name="pst")
                        nc.tensor.matmul(pst, wtile[:, ts(hc, P)], ident)
                        # rows of lhsT are H-dim -> scale row h by norm_w[h]
                        nc.vector.tensor_scalar_mul(
                            winT_sb[hc][:, ts(oc, P)], pst, nw_col[hc]
                        )
                for hc in range(HC):
                    nc.sync.dma_start(w_inT[li, hc], winT_sb[hc])
                # out_proj: want lhsT [I, H] = out_w.T
                woutT_sb = [pwb.tile([P, H], BF16, name=f"woutT_sb{c}") for c in range(ICN)]
                for hc in range(HC):
                    wtile = pw.tile([P, I], FP32, name="w_ld2")
                    nc.sync.dma_start(wtile, out_w[li, ts(hc, P), :])
                    for ic in range(ICN):
                        pst = ppt.tile([P, P], FP32, name="pst")
                        nc.tensor.matmul(pst, wtile[:, ts(ic, P)], ident)
                        nc.scalar.copy(woutT_sb[ic][:, ts(hc, P)], pst)
                for ic in range(ICN):
                    nc.sync.dma_start(w_outT[li, ic], woutT_sb[ic])
                # x_proj: want lhsT [I, 48] = xp_w.T (padded layout 0:16,32:48,64:80)
                xp_sb = pw.tile([R + 2 * S, I], FP32, name="w_ld2")
                nc.sync.dma_start(xp_sb, xp_w[li])
                for ic in range(ICN):
                    pst = ppt.tile([P, P], FP32, name="pst")
                    nc.tensor.matmul(
                        pst[:, : R + 2 * S], xp_sb[:, ts(ic, P)],
                        ident[: R + 2 * S, : R + 2 * S],
                    )
                    wx = pw.tile([P, XP80], BF16, name="wx")
                    nc.vector.memset(wx, 0.0)
                    nc.scalar.copy(wx[:, :R], pst[:, :R])        # dt rows 0:16
                    nc.scalar.copy(wx[:, 32:48], pst[:, R : R + S])       # B -> 32:48
                    nc.scalar.copy(wx[:, 64:80], pst[:, R + S : R + 2 * S])  # C -> 64:80
                    nc.sync.dma_start(w_xpT[li, ic], wx)
                # dt_proj: want lhsT [R+1, I]: rows 0..R-1 = dt_w.T, row R = dt_b
                wdt32 = pwb.tile([R + 1, I], FP32, name="wdt32")
                for ic in range(ICN):
                    wtile = pw.tile([P, R], FP32, name="w_ld3")
                    nc.sync.dma_start(wtile, dt_w[li, ts(ic, P), :])
                    pst = ppt.tile([P, P], FP32, name="pst")
                    nc.tensor.matmul(pst[:R], wtile, ident)
                    nc.scalar.copy(wdt32[:R, ts(ic, P)], pst[:R])
                nc.sync.dma_start(wdt32[R : R + 1, :], dt_b[li][None, :])
                wdt_sb = pwb.tile([R + 1, I], BF16, name="wdt_sb")
                nc.vector.tensor_copy(wdt_sb, wdt32)
                nc.sync.dma_start(w_dtT[li], wdt_sb)

            # ================= layers =================
            for li in range(n_layers):
                # per-layer small tensors
                w_in_sb = [pl.tile([P, 2 * I], BF16, name=f"w_in{h}") for h in range(HC)]
                for hc in range(HC):
                    nc.sync.dma_start(w_in_sb[hc], w_inT[li, hc])
                w_out_sb = [pl.tile([P, H], BF16, name=f"w_out{c}") for c in range(ICN)]
                w_xp_sb = [pl.tile([P, XP80], BF16, name=f"w_xp{c}") for c in range(ICN)]
                for ic in range(ICN):
                    nc.sync.dma_start(w_out_sb[ic], w_outT[li, ic])
                    nc.sync.dma_start(w_xp_sb[ic], w_xpT[li, ic])
                w_dt_sb = pl.tile([R + 1, I], BF16, name="w_dt")
                nc.sync.dma_start(w_dt_sb, w_dtT[li])
                cb_sb = [pl.tile([P, 1], FP32, name=f"cb{c}") for c in range(ICN)]
                a_neg = [pl.tile([P, S], FP32, name=f"an{c}") for c in range(ICN)]
                a_bias = [pl.tile([P, S], FP32, name=f"ab{c}") for c in range(ICN)]
                # conv weights as diagonal [P,P] bf16 matmul operands
                cdiag = [
                    [pl.tile([P, P], BF16, name=f"cd{c}_{k}") for k in range(KCONV)]
                    for c in range(ICN)
                ]
                ddiag = [pl.tile([P, P], BF16, name=f"dd{c}") for c in range(ICN)]
                for ic in range(ICN):
                    cw_ld = pt.tile([P, KCONV], FP32, name="cw_ld")
                    nc.sync.dma_start(cw_ld, conv_w[li, ts(ic, P), :])
                    for k in range(KCONV):
                        nc.vector.tensor_scalar_mul(
                            cdiag[ic][k], ident_bf, cw_ld[:, k : k + 1]
                        )
                    d_ld = pt.tile([P, 1], FP32, name="d_ld")
                    nc.sync.dma_start(d_ld, D_in[li, ts(ic, P)][:, None])
                    nc.vector.tensor_scalar_mul(ddiag[ic], ident_bf, d_ld)
                    nc.sync.dma_start(cb_sb[ic], conv_b[li, ts(ic, P)][:, None])
                    atile = pt.tile([P, S], FP32, name="a_ld")
                    nc.sync.dma_start(atile, A_log[li, ts(ic, P), :])
                    nc.scalar.activation(a_neg[ic], atile, AF.Exp)
                    nc.vector.tensor_scalar_mul(a_neg[ic], a_neg[ic], -1.0)
                    nc.vector.tensor_scalar_mul(a_bias[ic], a_neg[ic], 0.7)

                for b in range(BLOC):
                    # ---- load x (residual stream) in [H, L] layout ----
                    xT = [pl.tile([P, L], BF16, name=f"xT{h}") for h in range(HC)]
                    for hc in range(HC):
                        nc.sync.dma_start(xT[hc], xT_dram[b, hc])

                    # ---- rmsnorm (norm_w folded into in_proj weights) ----
                    hn = [pt.tile([P, L], BF16, name=f"hn{h}") for h in range(HC)]
                    for hc in range(HC):
                        nc.scalar.activation(hn[hc], xT[hc], AF.Square)
                    r16 = pt.tile([1, L], BF16, name="r16")
                    for nn in range(NN):
                        ms_ps = pp.tile([P, NT], FP32, name="psm")[:1]
                        for hc in range(HC):
                            nc.tensor.matmul(
                                ms_ps, ones_col, hn[hc][:, ts(nn, NT)],
                                start=(hc == 0), stop=(hc == HC - 1),
                            )
                        # r = (mean_sq + eps)^-0.5 = exp(-0.5*ln(ms/H + eps))
                        lnr = pt.tile([1, NT], FP32, name="lnr")
                        nc.scalar.activation(
                            lnr, ms_ps, AF.Ln, bias=eps_col[:1], scale=1.0 / H
                        )
                        nc.scalar.activation(r16[:, ts(nn, NT)], lnr, AF.Exp, scale=-0.5)
                    nc.sync.dma_start(r_dram.ap(), r16)
                    r_rep = pt.tile([P, L], BF16, name="r_rep")
                    nc.sync.dma_start(r_rep, r_dram.ap().to_broadcast((P, L)))
                    for hc in range(HC):
                        nc.vector.tensor_copy(hn[hc], xT[hc])
                        nc.vector.tensor_tensor(hn[hc], hn[hc], r_rep, op=OP.mult)

                    # ---- in_proj ----
                    hs_pad = [pl.tile([P, KCONV - 1 + L], BF16, name=f"hsp{c}") for c in range(ICN)]
                    sg = [pl.tile([P, L], BF16, name=f"sg{c}") for c in range(ICN)]
                    for ic in range(ICN):
                        nc.vector.memset(hs_pad[ic][:, 0 : KCONV - 1], 0.0)
                    for oc in range(OCN):
                        for nn in range(NN):
                            psm = pp.tile([P, NT], FP32, name="psm")
                            for hc in range(HC):
                                nc.tensor.matmul(
                                    psm, w_in_sb[hc][:, ts(oc, P)], hn[hc][:, ts(nn, NT)],
                                    start=(hc == 0), stop=(hc == HC - 1),
                                )
                            if oc < ICN:
                                nc.scalar.copy(
                                    hs_pad[oc][:, KCONV - 1 + nn * NT : KCONV - 1 + (nn + 1) * NT],
                                    psm,
                                )
                            else:
                                nc.scalar.activation(
                                    sg[oc - ICN][:, ts(nn, NT)], psm, AF.Silu
                                )

                    # ---- depthwise causal conv (tensor engine) + silu ----
                    u = [pl.tile([P, L], BF16, name=f"u{c}") for c in range(ICN)]
                    for ic in range(ICN):
                        for nn in range(NN):
                            pcv = pp.tile([P, NT], FP32, name="psm")
                            for k in range(KCONV):
                                nc.tensor.matmul(
                                    pcv, cdiag[ic][k],
                                    hs_pad[ic][:, k + nn * NT : k + (nn + 1) * NT],
                                    start=(k == 0), stop=(k == KCONV - 1),
                                )
                            nc.scalar.activation(
                                u[ic][:, ts(nn, NT)], pcv, AF.Silu, bias=cb_sb[ic]
                            )

                    # ---- x_proj (fused with dt_proj) ----
                    dtp = [pl.tile([P, L], BF16, name=f"dtp{c}") for c in range(ICN)]
                    for nn in range(NN):
                        ps48_f = pp.tile([P, NT], FP32, name="psm")
                        ps48 = ps48_f[:XP80]
                        for ic in range(ICN):
                            nc.tensor.matmul(
                                ps48, w_xp_sb[ic], u[ic][:, ts(nn, NT)],
                                start=(ic == 0), stop=(ic == ICN - 1),
                            )
                        dtr_nn = pt.tile([R + 1, NT], BF16, name="dtr_nn")
                        nc.vector.memset(dtr_nn, 1.0)  # row R = ones (bias row)
                        nc.scalar.copy(dtr_nn[0:R], ps48[0:R])
                        bt = pt.tile([S, NT], BF16, name="bt")
                        nc.scalar.copy(bt, ps48[32:48])
                        nc.sync.dma_start(B_dram.ap()[:, ts(nn, NT)], bt)
                        ct = pt.tile([S, NT], BF16, name="ct")
                        nc.scalar.copy(ct, ps48[64:80])
                        nc.sync.dma_start(C_dram.ap()[:, ts(nn, NT)], ct)
                        for mc in range(ICN):
                            psd = pp.tile([P, NT], FP32, name="psm")
                            nc.tensor.matmul(psd, w_dt_sb[:, ts(mc, P)], dtr_nn)
                            dt32 = pt.tile([P, NT], FP32, name="dt32")
                            nc.scalar.activation(dt32, psd, AF.Exp)
                            # softplus(x) = ln(exp(x) + 1); store centered at -0.7
                            nc.scalar.activation(dt32, dt32, AF.Ln, bias=one_col)
                            nc.vector.tensor_scalar_add(dtp[mc][:, ts(nn, NT)], dt32, -0.7)

                    # ---- dtu = dt * u ----
                    dtu = [pl.tile([P, L], BF16, name=f"dtu{c}") for c in range(ICN)]
                    for mc in range(ICN):
                        nc.vector.scalar_tensor_tensor(
                            dtu[mc], dtp[mc], 0.7, u[mc], op0=OP.add, op1=OP.mult
                        )

                    # ---- SSM scan (full-L), y accumulated on PE in PSUM ----
                    y_ssm = [pl.tile([P, L], BF16, name=f"yss{c}") for c in range(ICN)]
                    for ic in range(ICN):
                        yps = ppy.tile([P, L], FP32, name="yps")
                        for nn in range(NN):
                            # D-skip term starts the accumulation
                            nc.tensor.matmul(
                                yps[:, ts(nn, NT)], ddiag[ic], u[ic][:, ts(nn, NT)],
                                start=True, stop=False,
                            )
                        for s in range(S):
                            B_rep = pb.tile([P, L], BF16, name="B_rep")
                            nc.sync.dma_start(
                                B_rep, B_dram.ap()[s : s + 1, :].to_broadcast((P, L))
                            )
                            C_rep = pb.tile([P, L], BF16, name="C_rep")
                            nc.sync.dma_start(
                                C_rep, C_dram.ap()[s : s + 1, :].to_broadcast((P, L))
                            )
                            dA = ps.tile([P, L], BF16, name="dA")
                            nc.scalar.activation(
                                dA, dtp[ic], AF.Exp,
                                bias=a_bias[ic][:, s : s + 1],
                                scale=a_neg[ic][:, s : s + 1],
                            )
                            xt = ps.tile([P, L], BF16, name="xt")
                            nc.vector.tensor_tensor(xt, dtu[ic], B_rep, op=OP.mult)
                            hscan = ps.tile([P, L], BF16, name="hscan")
                            nc.vector.tensor_tensor_scan(
                                hscan, dA, xt, 0.0, op0=OP.mult, op1=OP.add
                            )
                            hC = ps.tile([P, L], BF16, name="hC")
                            nc.vector.tensor_tensor(hC, hscan, C_rep, op=OP.mult)
                            for nn in range(NN):
                                nc.tensor.matmul(
                                    yps[:, ts(nn, NT)], ident_bf, hC[:, ts(nn, NT)],
                                    start=False, stop=(s == S - 1),
                                )
                        # gate: y_ssm = ypsum * silu(gate)
                        nc.vector.tensor_tensor(y_ssm[ic], yps, sg[ic], op=OP.mult)

                    # ---- out_proj + residual (PE) ----
                    for hc in range(HC):
                        for nn in range(NN):
                            pso = pp.tile([P, NT], FP32, name="psm")
                            nc.tensor.matmul(
                                pso, ident_bf, xT[hc][:, ts(nn, NT)],
                                start=True, stop=False,
                            )
                            for ic in range(ICN):
                                nc.tensor.matmul(
                                    pso, w_out_sb[ic][:, ts(hc, P)], y_ssm[ic][:, ts(nn, NT)],
                                    start=False, stop=(ic == ICN - 1),
                                )
                            nc.scalar.copy(xT[hc][:, ts(nn, NT)], pso)
                    for hc in range(HC):
                        nc.sync.dma_start(xT_dram[b, hc], xT[hc])

            # ---- transpose x back to [L, H] and write out ----
            for b in range(BLOC):
                for tc_i in range(L // P):
                    o_sb = pw.tile([P, H], FP32, name="xio_small")
                    for hc in range(HC):
                        xf = pw.tile([P, P], BF16, name="xio_f")
                        nc.sync.dma_start(xf, xT_dram[b, hc][:, ts(tc_i, P)])
                        pst = ppt.tile([P, P], FP32, name="pst")
                        nc.tensor.matmul(pst, xf, ident_bf)
                        nc.vector.tensor_copy(o_sb[:, ts(hc, P)], pst)
                    nc.sync.dma_start(y_out[b, ts(tc_i, P), :], o_sb)

    return nc


def _split_matmul_waits(nc):
    """walrus codegen allows limited sync waits per instruction;
    hoist extras into EventSemaphore instructions on the same engine."""
    ctr = 0
    for fn in nc.m.functions:
        for bb in fn.blocks:
            insts = bb.instructions
            out = []
            changed = False
            for inst in insts:
                si = inst.sync_info
                if (
                    not isinstance(inst, mybir.InstEventSemaphore)
                    and si is not None
                    and si.on_wait
                    and len(si.on_wait) > 1
                ):
                    waits = list(si.on_wait)
                    for w in waits[: -1]:
                        ev = mybir.InstEventSemaphore(
                            name=f"I-mmwait-{ctr}",
                            engine=inst.engine,
                            sync_info=mybir.SyncInfo(on_wait=[w], on_update=[]),
                            ins=[],
                            outs=[],
                        )
                        ctr += 1
                        out.append(ev)
                    inst.sync_info = mybir.SyncInfo(
                        on_wait=[waits[-1]], on_update=list(si.on_update or [])
                    )
                    changed = True
                out.append(inst)
            if changed:
                bb.instructions = out
    return nc


def kernel(**inputs):
    from concourse.bass_utils import run_bass_kernel_spmd

    x = np.asarray(inputs["x"], dtype=np.float32)
    Bfull, L, _ = x.shape
    nc = build_program(L=L, n_layers=NL)
    _split_matmul_waits(nc)

    weight_names = [
        "norm_w", "in_proj_w", "conv_w", "conv_b", "x_proj_w",
        "dt_proj_w", "dt_proj_b", "A_log", "D", "out_proj_w",
    ]
    weights = {k: np.asarray(inputs[k], dtype=np.float32) for k in weight_names}

    in_maps = []
    for c in range(NCORES):
        m = {"x": x[c * BLOC : (c + 1) * BLOC]}
        m.update(weights)
        in_maps.append(m)

    res = run_bass_kernel_spmd(nc, in_maps, core_ids=list(range(NCORES)))
    out = np.concatenate([r["out"] for r in res.results], axis=0)
    return out
